# revision 24
# baseline (speedup 1.0000x reference)
"""Trainium2 Bass kernel for nn_LunaCausalAttention.

Sharding: 8 cores; core c handles batch b = c//4 and heads hs = 4*(c%4) .. hs+4.
Each core computes its 4 heads' projections (feature-major bf16 matmuls), a
chunked two-pass causal linear attention (C=128, head pairs packed into the
128-partition dim), and a partial output projection over its 256 head-features.
Host sums the 4 partials per batch and adds the output bias.

Structure (vs the naive per-chunk serial loop):
  - Z = softplus(beta*pattn) for ALL chunks is computed upfront with wide
    Activation ops (native Softplus, so act tables load exactly twice:
    softplus-set during the Z phase, exp-set for the softmax phase).
  - Cross-chunk state (S = cumsum K^T Z, T = cumsum Z^T V) is computed as
    independent per-chunk partials on PE, then prefix-summed on DVE/Pool,
    removing the serial PE->Act->DVE->PE chain between chunks.
  - The 16 (chunk, head-pair) softmax iterations are then fully independent
    and software-pipelined across 5 PE stages so PE never blocks on the
    Act/DVE/Pool helper ops of the same iteration.
"""
import numpy as np

import concourse.bass as bass
import concourse.mybir as mybir
import concourse.tile as tile
from concourse import bacc
from concourse.masks import make_upper_triangular, make_identity
from concourse.bass_utils import run_bass_kernel_spmd

# static shapes
B, N, D, M, H, DH = 2, 1024, 1024, 64, 16, 64
C = 128                 # token chunk
NCH = N // C            # 8 chunks
NCORES = 8
HPC = 4                 # heads per core
E = HPC * DH            # 256 per-core head features
NF = D // 128           # 8 contraction tiles
BETA = float(np.log(2.0))
SCALE = DH ** -0.5

F32 = mybir.dt.float32
BF16 = mybir.dt.bfloat16
ADT = BF16              # attention-core operand dtype
AF = mybir.ActivationFunctionType


def build_bass(phase=3):
    nc = bacc.Bacc(None, target_bir_lowering=False)

    # ---- I/O ----
    xT_d = nc.dram_tensor("xT", [D, N], BF16, kind="ExternalInput")       # query[b].T
    pT_d = nc.dram_tensor("pT", [D, M], BF16, kind="ExternalInput")       # p[b].T
    wq_d = nc.dram_tensor("wq", [D, E], BF16, kind="ExternalInput")       # scale folded
    wk_d = nc.dram_tensor("wk", [D, E], BF16, kind="ExternalInput")
    wv_d = nc.dram_tensor("wv", [D, E], BF16, kind="ExternalInput")
    wpc_d = nc.dram_tensor("wpc", [D, E], BF16, kind="ExternalInput")
    wpq_d = nc.dram_tensor("wpq", [D, E], BF16, kind="ExternalInput")     # scale folded
    wo_d = nc.dram_tensor("wo", [E, D], BF16, kind="ExternalInput")
    # all small per-core constants batched into two DMAs:
    # bcat cols: bq(2) bk(2) bpc(2) bpq(2) rc(8)  [each b col2-packed]
    bcat_d = nc.dram_tensor("bcat", [128, 16], F32, kind="ExternalInput")
    # rcat cols: ones(128) bvr(E)
    rcat_d = nc.dram_tensor("rcat", [1, 128 + E], BF16, kind="ExternalInput")
    out_d = nc.dram_tensor("outp", [N, D], F32, kind="ExternalOutput")

    with tile.TileContext(nc) as tc:
        with (
            tc.tile_pool(name="singles", bufs=1) as singles,
            tc.tile_pool(name="work", bufs=4) as work,
            tc.tile_pool(name="obuf", bufs=3) as obuf,
        ):
            # ---- constants (engine-generated, no DMA) ----
            triu = singles.tile([128, 2 * C], F32)      # two upper-tri copies
            make_upper_triangular(nc, triu[:, 0:C], val=1.0, diag=True)
            make_upper_triangular(nc, triu[:, C:2 * C], val=1.0, diag=True)
            identb = singles.tile([128, 128], ADT)
            make_identity(nc, identb)

            # ---- input DMAs ----
            # small constants: two batched DMAs on the Pool/SWDGE queue so
            # they don't delay the critical SP-queue weight loads
            bcat_sb = singles.tile([128, 16], F32)
            nc.gpsimd.dma_start(out=bcat_sb, in_=bcat_d[:, :])
            bq_sb = bcat_sb[:, 0:2]
            bk_sb = bcat_sb[:, 2:4]
            bpc_sb = bcat_sb[:, 4:6]
            bpq_sb = bcat_sb[:, 6:8]
            rc_sb = bcat_sb[:, 8:16]
            rcat_sb = singles.tile([1, 128 + E], BF16)
            nc.gpsimd.dma_start(out=rcat_sb, in_=rcat_d[:, :])
            ones = rcat_sb[:, 0:128]
            bvr_sb = rcat_sb[:, 128:128 + E]
            pT_sb = singles.tile([128, NF, M], BF16)
            nc.sync.dma_start(
                out=pT_sb, in_=pT_d.rearrange("(f p) m -> p f m", p=128))

            def load_w(name, dram, halves=1):
                w = singles.tile([128, NF, E], BF16, name=name)
                hf = NF // halves
                for i in range(halves):
                    nc.sync.dma_start(
                        out=w[:, i * hf:(i + 1) * hf, :],
                        in_=dram.rearrange("(f p) e -> p f e", p=128)
                        [:, i * hf:(i + 1) * hf, :])
                return w

            wpc_sb = load_w("wpc_sb", wpc_d, halves=2)
            wpq_sb = load_w("wpq_sb", wpq_d)
            # xt tiles on the Activation HWDGE queue: issues in parallel with
            # the SP-queue weight loads (Act is idle this early)
            xt_sb = []
            for f in range(NF):
                xt = singles.tile([128, N], BF16, name=f"xt{f}")
                for g in range(2):
                    nc.scalar.dma_start(
                        out=xt[:, g * 512:(g + 1) * 512],
                        in_=xT_d[f * 128:(f + 1) * 128,
                                 g * 512:(g + 1) * 512])
                xt_sb.append(xt)
            wq_sb = load_w("wq_sb", wq_d, halves=2)
            wk_sb = load_w("wk_sb", wk_d, halves=2)
            wv_sb = load_w("wv_sb", wv_d)
            wo_sb = singles.tile([128, 2, D], BF16)
            nc.sync.dma_start(
                out=wo_sb, in_=wo_d.rearrange("(t p) o -> p t o", p=128))

            # ---- persistent SBUF tensors ----
            qT_sb = singles.tile([128, 2, N], ADT)    # [:, et, t] feature-major
            kT_sb = singles.tile([128, 2, N], ADT)
            pcT_sb = singles.tile([128, 2, N], ADT)
            pq_sb = singles.tile([128, 2, M], ADT)
            # Z token-major: [:, hp, h, c, m]
            zt_sb = singles.tile([128, 2, 2, NCH, M], ADT)
            # Z m-major (Z^T): [m, c, hp, h, tok]
            at_sb = singles.tile([64, NCH, 2, 2, C], ADT)
            # K token-major: [tok, c, hp, featpair]
            ktok_sb = singles.tile([128, NCH, 2, C], ADT)
            vtok_sb = [singles.tile([128, E], ADT, name=f"vtok{t}")
                       for t in range(NCH)]
            attnT_sb = [singles.tile([128, 2, C], ADT, name=f"attnT{t}")
                        for t in range(NCH)]
            # per-chunk state partials (SBUF copies) and their prefix sums:
            # S [featpair, hp, c, m];  T [m, hp, c, 2h*dh]
            Sp_sb = singles.tile([128, 2, NCH, M], ADT)
            Tp_sb = singles.tile([64, 2, NCH, 2 * DH], ADT)
            Spref_sb = singles.tile([128, 2, NCH, M], ADT)
            Tpref_sb = singles.tile([64, 2, NCH, 2 * DH], ADT)

            lo, hi = slice(0, 64), slice(64, 128)
            sls = (lo, hi)

            # ================= early phase (own PSUM pool) =================
            with tc.tile_pool(name="psum_e", bufs=1, space="PSUM") as psum_e:
                # ---- feature-major projection helper ----
                def proj_et(dst, w, b, et):
                    for nh in range(2):
                        pp = psum_e.tile([128, 512], F32, tag="pp", bufs=2,
                                         name="pp")
                        for f in range(NF):
                            nc.tensor.matmul(
                                pp, w[:, f, et * 128:(et + 1) * 128],
                                xt_sb[f][:, nh * 512:(nh + 1) * 512],
                                start=(f == 0), stop=(f == NF - 1))
                        nc.scalar.activation(
                            dst[:, et, nh * 512:(nh + 1) * 512], pp,
                            AF.Identity, bias=b[:, et:et + 1])

                proj_et(pcT_sb, wpc_sb, bpc_sb, 0)
                proj_et(pcT_sb, wpc_sb, bpc_sb, 1)

                # ---- pq projection: (128, M) pair tiles ----
                for et in range(2):
                    ppq = psum_e.tile([128, 512], F32, tag="pp", bufs=2,
                                      name="ppq")
                    for f in range(NF):
                        nc.tensor.matmul(
                            ppq[:, 0:M], wpq_sb[:, f, et * 128:(et + 1) * 128],
                            pT_sb[:, f, :],
                            start=(f == 0), stop=(f == NF - 1))
                    nc.vector.tensor_scalar_add(pq_sb[:, et, :], ppq[:, 0:M],
                                                bpq_sb[:, et:et + 1])


                # ---- Z token-major: pattn psum banks by (hp, h), then
                # softplus as Exp + Ln(1+x) (real act tables lack softplus;
                # both live in natural_log_exp_and_others = 1 table load) ----
                for hp in range(2):
                    for h in range(2):
                        s = sls[h]
                        pz = psum_e.tile([128, 512], F32, tag="pz", bufs=2,
                                         name="pz")
                        for c in range(NCH):
                            nc.tensor.matmul(
                                pz[:, c * M:(c + 1) * M],
                                pcT_sb[s, hp, c * C:(c + 1) * C],
                                pq_sb[s, hp, :], start=True, stop=True,
                                tile_position=(64 * h, 0))
                        ez = work.tile([128, 512], F32, name="ez", bufs=2)
                        nc.scalar.activation(ez, pz, AF.Exp, scale=BETA)
                        nc.scalar.activation(zt_sb[:, hp, h, :, :], ez,
                                             AF.Ln, bias=1.0)

                proj_et(qT_sb, wq_sb, bq_sb, 0)
                proj_et(qT_sb, wq_sb, bq_sb, 1)
                proj_et(kT_sb, wk_sb, bk_sb, 0)
                proj_et(kT_sb, wk_sb, bk_sb, 1)

                # ---- K_tok: PE transposes of kT, batched copies ----
                # bank holds 8 (c,hp) transposes of [128, 128]bf16 (=64 f32)
                for half in range(2):   # chunks 0-3 / 4-7
                    pz = psum_e.tile([128, 512], F32, tag="pz", bufs=2,
                                     name="pkt")
                    pv = pz.bitcast(ADT).rearrange("p (c k t) -> p c k t",
                                                   c=4, k=2)
                    for c4 in range(4):
                        c = half * 4 + c4
                        for hp in range(2):
                            nc.tensor.transpose(
                                pv[:, c4, hp, :],
                                kT_sb[:, hp, c * C:(c + 1) * C], identb)
                    nc.scalar.activation(
                        ktok_sb[:, half * 4:half * 4 + 4, :, :], pv, AF.Copy)

                # ---- Z^T: PE transposes of zt, batched copies ----
                # bank holds 8 strips of [64, 128]bf16: (c, hp, h) for 2 c's
                for half in range(4):   # chunk pairs
                    pz = psum_e.tile([128, 512], F32, tag="pz", bufs=2,
                                     name="pzt")
                    pv = pz.bitcast(ADT).rearrange(
                        "p (c k h t) -> p c k h t", c=2, k=2, h=2)
                    for c2 in range(2):
                        c = half * 2 + c2
                        for hp in range(2):
                            for h in range(2):
                                nc.tensor.transpose(
                                    pv[0:64, c2, hp, h, :],
                                    zt_sb[:, hp, h, c, :], identb)
                    nc.scalar.activation(
                        at_sb[:, half * 2:half * 2 + 2, :, :, :],
                        pv[0:64, :, :, :, :], AF.Copy)

                # ---- V_tok: token-major projection, 2 chunks per bank ----
                for half in range(4):
                    pv = psum_e.tile([128, 512], F32, tag="pp", bufs=2,
                                     name="pv")
                    for c2 in range(2):
                        c = half * 2 + c2
                        for f in range(NF):
                            nc.tensor.matmul(
                                pv[:, c2 * E:(c2 + 1) * E],
                                xt_sb[f][:, c * 128:(c + 1) * 128],
                                wv_sb[:, f, :], start=(f == 0), stop=False)
                        nc.tensor.matmul(pv[:, c2 * E:(c2 + 1) * E], ones,
                                         bvr_sb, start=False, stop=True)
                    nc.vector.tensor_copy(vtok_sb[half * 2], pv[:, 0:E])
                    nc.vector.tensor_copy(vtok_sb[half * 2 + 1], pv[:, E:2 * E])

                # ---- state partials: batch to SBUF, then bf16 2x-mode
                # prefix chains on DVE (all-SBUF so they pipeline with
                # phase A instead of holding PSUM banks hostage) ----
                # S partials: [featpair, c, m] one bank per hp
                for hp in range(2):
                    ps = psum_e.tile([128, 512], F32, tag="psS", bufs=2,
                                     name="psS")
                    psv = ps.rearrange("p (c m) -> p c m", c=NCH)
                    for c in range(NCH):
                        for h in range(2):
                            nc.tensor.matmul(
                                psv[64 * h:64 * h + 64, c, :],
                                ktok_sb[:, c, hp, 64 * h:64 * h + 64],
                                zt_sb[:, hp, h, c, :],
                                start=True, stop=True,
                                tile_position=(0, 64 * h))
                    nc.vector.tensor_copy(Sp_sb[:, hp, :, :], psv)
                    # S prefix chain immediately (bf16 all-SBUF, DVE 2x)
                    nc.vector.tensor_copy(Spref_sb[:, hp, 1, :],
                                          Sp_sb[:, hp, 0, :])
                    for c in range(2, NCH):
                        nc.vector.tensor_add(
                            Spref_sb[:, hp, c, :], Spref_sb[:, hp, c - 1, :],
                            Sp_sb[:, hp, c - 1, :])

                # T partials: [m, c, 2h*dh]; 4 chunks per bank
                for hp in range(2):
                    for half in range(2):
                        pt = psum_e.tile([64, 512], F32, tag="psT", bufs=2,
                                         name="psT")
                        ptv = pt.rearrange("p (c d) -> p c d", c=4)
                        for c4 in range(4):
                            c = half * 4 + c4
                            for h in range(2):
                                nc.tensor.matmul(
                                    ptv[:, c4, 64 * h:64 * h + 64],
                                    zt_sb[:, hp, h, c, :],
                                    vtok_sb[c][:, hp * 128 + 64 * h:
                                               hp * 128 + 64 * h + 64],
                                    start=True, stop=True,
                                    tile_position=(0, 0))
                        nc.scalar.activation(
                            Tp_sb[:, hp, half * 4:half * 4 + 4, :], ptv,
                            AF.Copy)
                    # T prefix chain immediately
                    nc.vector.tensor_copy(Tpref_sb[:, hp, 1, :],
                                          Tp_sb[:, hp, 0, :])
                    for c in range(2, NCH):
                        nc.vector.tensor_add(
                            Tpref_sb[:, hp, c, :], Tpref_sb[:, hp, c - 1, :],
                            Tp_sb[:, hp, c - 1, :])

            # ================= attention phase (pipelined) =================
            iters = [(c, hp) for c in range(NCH) for hp in range(2)]
            NIT = len(iters)
            state = [dict() for _ in range(NIT)]
            out_jobs = []

            with tc.tile_pool(name="psum_a", bufs=1, space="PSUM") as psum:

                def stage_G(i):
                    c, hp = iters[i]
                    st = state[i]
                    tok = slice(c * C, (c + 1) * C)
                    # pa: G h0 (0:128) + aw h0|h1 (128:256).  All aw matmuls
                    # are full-height or separated by a full-height one, so
                    # their drains never run concurrently -> one bank is safe.
                    # G h1 row-tiles concurrently with G h0 -> own bank.
                    pa = psum.tile([128, 256], F32, tag="pHA", bufs=2,
                                   name="pa")
                    pb = psum.tile([128, 128], F32, tag="pHB", bufs=2,
                                   name="pb")
                    st["pab"] = (pa, pb)
                    gm = work.tile([128, 256], ADT, name="gm")
                    st["gm"] = gm
                    for h in range(2):
                        p = (pa, pb)[h]
                        nc.tensor.matmul(p[:, 0:128], kT_sb[sls[h], hp, tok],
                                         qT_sb[sls[h], hp, tok], start=True,
                                         stop=True, tile_position=(64 * h, 0))
                        nc.vector.tensor_mul(gm[:, 128 * h:128 * h + 128],
                                             p[:, 0:128], triu[:, 0:C])

                def stage_aw(i):
                    c, hp = iters[i]
                    st = state[i]
                    tok = slice(c * C, (c + 1) * C)
                    gm = st["gm"]
                    paw = st["pab"][0]
                    ex = work.tile([128, 2, 64], ADT, name="ex")
                    rs = work.tile([128, 2], F32, name="rs")
                    rcp = work.tile([128, 2], F32, name="rcp")
                    pt2 = work.tile([128, 128], ADT, name="pt2", bufs=6)
                    st["pt2"] = pt2
                    for h in range(2):
                        nc.tensor.matmul(paw[:, 128 + 64 * h:192 + 64 * h],
                                         gm[:, 128 * h:128 * h + 128],
                                         zt_sb[:, hp, h, c, :],
                                         start=True, stop=(c == 0))
                        if c > 0:
                            nc.tensor.matmul(paw[:, 128 + 64 * h:192 + 64 * h],
                                             qT_sb[sls[h], hp, tok],
                                             Spref_sb[sls[h], hp, c, :],
                                             start=False, stop=True,
                                             tile_position=(64 * h, 0))
                    nc.scalar.activation(ex, paw[:, 128:256], AF.Exp,
                                         scale=rc_sb[:, c:c + 1])
                    nc.vector.tensor_reduce(rs, ex, mybir.AxisListType.X,
                                            mybir.AluOpType.add)
                    nc.vector.reciprocal(rcp, rs)
                    for h in range(2):
                        nc.gpsimd.tensor_scalar(
                            pt2[:, 64 * h:64 * h + 64], ex[:, h, :],
                            rcp[:, h:h + 1], rc_sb[:, c:c + 1],
                            mybir.AluOpType.mult, mybir.AluOpType.mult)

                def stage_trans(i):
                    st = state[i]
                    pm = psum.tile([128, 512], F32, tag="pMD", bufs=3,
                                   name="pm")
                    st["pm"] = pm
                    ptr2 = pm[0:64, 256:384].bitcast(ADT).rearrange(
                        "p (h t) -> p h t", h=2)
                    st["ptr2"] = ptr2
                    ptT = work.tile([64, 2, 128], ADT, name="ptT", bufs=6)
                    st["ptT"] = ptT
                    pt2 = st["pt2"]
                    for h in range(2):
                        nc.tensor.transpose(ptr2[:, h, :],
                                            pt2[:, 64 * h:64 * h + 64],
                                            identb, tile_position=(0, 0))
                    nc.scalar.activation(ptT, ptr2, AF.Copy)

                def stage_pm(i):
                    c, hp = iters[i]
                    st = state[i]
                    pm = st["pm"]
                    ptT = st["ptT"]
                    g2m = work.tile([128, 256], ADT, name="g2m")
                    st["g2m"] = g2m
                    for h in range(2):
                        nc.tensor.matmul(pm[:, 128 * h:128 * h + 128],
                                         at_sb[:, c, hp, h, :],
                                         ptT[:, h, :],
                                         start=True, stop=True,
                                         tile_position=(0, 0))
                    nc.vector.tensor_mul(g2m, pm[:, 0:256], triu)

                def stage_attn(i):
                    c, hp = iters[i]
                    st = state[i]
                    pm = st["pm"]
                    ptT = st["ptT"]
                    g2m = st["g2m"]
                    pan = pm[:, 384:512]
                    for h in range(2):
                        nc.tensor.matmul(
                            pan[64 * h:64 * h + 64, :],
                            vtok_sb[c][:, hp * 128 + 64 * h:
                                       hp * 128 + 64 * h + 64],
                            g2m[:, 128 * h:128 * h + 128],
                            start=True, stop=(c == 0),
                            tile_position=(0, 64 * h))
                        if c > 0:
                            nc.tensor.matmul(
                                pan[64 * h:64 * h + 64, :],
                                Tpref_sb[:, hp, c, 64 * h:64 * h + 64],
                                ptT[:, h, :],
                                start=False, stop=True,
                                tile_position=(0, 64 * h))
                    nc.scalar.activation(attnT_sb[c][:, hp, :], pan, AF.Copy)
                    if hp == 1:
                        out_jobs.append((c, 0))
                        out_jobs.append((c, 1))

                def out_job():
                    if not out_jobs:
                        return
                    c, oh = out_jobs.pop(0)
                    tok = slice(c * C, (c + 1) * C)
                    po = psum.tile([128, 512], F32, tag="pO", bufs=1,
                                   name="po")
                    for et in range(2):
                        nc.tensor.matmul(
                            po, attnT_sb[c][:, et, :],
                            wo_sb[:, et, oh * 512:(oh + 1) * 512],
                            start=(et == 0), stop=(et == 1))
                    ob = obuf.tile([128, 512], F32, name="ob", bufs=3)
                    if oh == 0:
                        nc.scalar.activation(ob, po, AF.Copy)
                    else:
                        nc.vector.tensor_copy(ob, po)
                    nc.sync.dma_start(
                        out=out_d[tok, oh * 512:(oh + 1) * 512], in_=ob)

                # software pipeline: stages offset by 1 slot each
                for s in range(NIT + 4):
                    if s < NIT:
                        stage_G(s)
                    if 1 <= s < NIT + 1:
                        stage_aw(s - 1)
                    if 2 <= s < NIT + 2:
                        stage_trans(s - 2)
                    if 3 <= s < NIT + 3:
                        stage_pm(s - 3)
                    if 4 <= s < NIT + 4:
                        stage_attn(s - 4)
                        out_job()
                # drain remaining out-proj jobs
                while out_jobs:
                    out_job()

    # Pin Exp and Ln to the combined natural_log_exp_and_others set so the
    # load-placement pass emits a single table load for the whole kernel.
    import concourse.bacc as _bacc_mod
    from concourse.hw_specs import get_activation_tables as _gat
    _orig_gat = _bacc_mod.get_activation_tables

    def _patched_gat(arch):
        t = _gat(arch)
        for name, s in t.items():
            if name != "natural_log_exp_and_others":
                s.discard(AF.Exp)
                s.discard(AF.Ln)
                s.discard(AF.Copy)
                s.discard(AF.Identity)
        return t

    _bacc_mod.get_activation_tables = _patched_gat
    try:
        nc.compile()
    finally:
        _bacc_mod.get_activation_tables = _orig_gat
    return nc


_CACHE = {}


import os


def _get_nc():
    phase = int(os.environ.get("KPHASE", "3"))
    key = f"nc{phase}"
    if key not in _CACHE:
        _CACHE[key] = build_bass(phase)
    return _CACHE[key]


def make_in_maps(query, p, Wq, bq, Wpq, bpq, Wpc, bpc, Wk, bk, Wv, bv, Wo, bo):
    import ml_dtypes
    bf = ml_dtypes.bfloat16
    f32 = lambda a: np.ascontiguousarray(np.asarray(a), dtype=np.float32)
    query, p = f32(query), f32(p)
    Wq, Wpq, Wpc, Wk, Wv, Wo = map(f32, (Wq, Wpq, Wpc, Wk, Wv, Wo))
    bq, bpq, bpc, bk, bv, bo = map(f32, (bq, bpq, bpc, bk, bv, bo))
    rc = (1.0 / ((np.arange(N) + 1.0) * BETA)).astype(np.float32)
    rc_cols = np.ascontiguousarray(rc.reshape(NCH, 128).T)

    def col2(v):  # (256,) -> (128, 2)
        return np.ascontiguousarray(v.reshape(2, 128).T)

    in_maps = []
    for core in range(NCORES):
        b = core // 4
        hs = (core % 4) * HPC
        cols = slice(hs * DH, (hs + HPC) * DH)
        bcat = np.concatenate(
            [col2(bq[cols] * SCALE), col2(bk[cols]), col2(bpc[cols]),
             col2(bpq[cols] * SCALE), rc_cols], axis=1)
        rcat = np.concatenate(
            [np.ones((1, 128), np.float32), bv[cols].reshape(1, E)], axis=1)
        m = {
            "xT": np.ascontiguousarray(query[b].T).astype(bf),
            "pT": np.ascontiguousarray(p[b].T).astype(bf),
            "wq": np.ascontiguousarray((Wq[cols, :] * SCALE).T).astype(bf),
            "wk": np.ascontiguousarray(Wk[cols, :].T).astype(bf),
            "wv": np.ascontiguousarray(Wv[cols, :].T).astype(bf),
            "wpc": np.ascontiguousarray(Wpc[cols, :].T).astype(bf),
            "wpq": np.ascontiguousarray((Wpq[cols, :] * SCALE).T).astype(bf),
            "wo": np.ascontiguousarray(Wo[:, cols].T).astype(bf),
            "bcat": np.ascontiguousarray(bcat, np.float32),
            "rcat": np.ascontiguousarray(rcat).astype(bf),
        }
        in_maps.append(m)
    return in_maps


def kernel(query, p, dec_input_mask=None, p_mask=None,
           Wq=None, bq=None, Wpq=None, bpq=None, Wpc=None, bpc=None,
           Wk=None, bk=None, Wv=None, bv=None, Wo=None, bo=None,
           _trace=False, _trace_kwargs=None):
    in_maps = make_in_maps(query, p, Wq, bq, Wpq, bpq, Wpc, bpc,
                           Wk, bk, Wv, bv, Wo, bo)
    res = run_bass_kernel_spmd(_get_nc(), in_maps, core_ids=list(range(NCORES)),
                               trace=_trace, **(_trace_kwargs or {}))
    out = np.zeros((B, N, D), np.float32)
    for core in range(NCORES):
        out[core // 4] += res.results[core]["outp"]
    out += np.asarray(bo, np.float32).reshape(1, 1, D)
    if _trace:
        kernel.last_result = res
    return out
